# revision 19
# baseline (speedup 1.0000x reference)
"""DBRX attention block (QKV proj + clamp + RoPE + GQA causal attention + out
proj) as a Bass/Tile kernel for 8 Trainium2 NeuronCores.

Problem shapes (hardcoded): B=2, S=2048, HID=2048, NH=16 q-heads, NKV=4 kv
heads, HD=128, clip +-8, rope theta 5e5.

Sharding: DP2 x TP4. Core c = (b = c//4, g = c%4) handles batch b with q-heads
4g..4g+3 and kv-head g (GQA group == core, so no KV duplication). Each core
computes a partial output [S, HID] = attn_flat[:, 512 cols] @ WoutT[512 rows];
the host sums the 4 partials per batch (tensor-parallel reduction).

Device-side layout trick: everything is computed transposed. The QKV matmul
produces qkv^T [o, t] directly (lhsT = Wqkv^T slice, rhs = hidden^T), which
puts head-dim on partitions - exactly what RoPE and the S^T = K^T.T @ Q^T
scores matmul want. Attention runs in S^T [k, q] layout: exp on ScalarE,
binary-mask multiply on the diagonal blocks, O^T accumulated via
lhsT=V[k-token, d], and softmax denominators via an all-ones lhsT matmul
(which also broadcasts the sums across all partitions for free). O^T [d, t]
feeds the output projection lhsT directly.

v2: the three phases are software-pipelined at emission level. Attention for
q-block j (whose PE work stalls on the ScalarE exp chain) is woven
instruction-group by instruction-group with independent dense matmul work -
the QKV projection of t-block j+1 and the output projection of q-block j-1.
This keeps the PE queue fed during exp latency (and keeps the HAM clock
warm), and spreads the ScalarE exp load across the whole kernel instead of
one ScalarE-bound middle phase. Output is stored + DMA'd as fp16 (halves
output traffic; host accumulates partials in f64).

All matmuls run in fp16 (scores <= 9.2 so exp <= 1e4 << 65504). Input DMA is
fp16 on three queues (sync=weights+out, gpsimd=activations, scalar=tables).
"""

import math
from contextlib import ExitStack

import numpy as np

import concourse.bacc as bacc
import concourse.bass as bass
import concourse.mybir as mybir
import concourse.tile as tile
from concourse.bass_utils import run_bass_kernel_spmd
from concourse.masks import make_identity

P = 128
B, S, HID = 2, 2048, 2048
NH, NKV, HD = 16, 4, 128
CLIP = 8.0
ROPE_THETA = 500000.0
NQ = NH // NKV        # q heads per core = 4
OC = NQ + 2           # o-chunks per core: 4 q heads, 1 k, 1 v
KC = HID // P         # 16 contraction chunks for qkv proj
TB = 512              # t-block (moving free dim) for qkv proj
NTB = S // TB         # 4
QB = 512              # q-block in attention
NQB = S // QB         # 4
NKB = S // P          # 16 k-blocks per sequence
DB = QB // P          # 4 diagonal 128-blocks per q-block
NCORES = 8

F16 = mybir.dt.float16
F32 = mybir.dt.float32

MM512 = 216           # ns per 512-wide fp16 matmul (warm)
MM128 = 85


def build_nc(reps: int = 1):
    nc = bacc.Bacc()

    hT = nc.dram_tensor("hT", [HID, S], F16, kind="ExternalInput")
    wqkvT = nc.dram_tensor("wqkvT", [HID, OC * P], F16, kind="ExternalInput")
    woutT = nc.dram_tensor("woutT", [NQ * P, HID], F16, kind="ExternalInput")
    cosT = nc.dram_tensor("cosT", [P, S], F16, kind="ExternalInput")
    sinT = nc.dram_tensor("sinT", [P, S], F16, kind="ExternalInput")
    out = nc.dram_tensor("out", [S, HID], F16, kind="ExternalOutput")

    hT3 = hT.rearrange("(kc p) t -> p kc t", p=P)          # [128, 16, 2048]
    wq3 = wqkvT.rearrange("(kc p) o -> p kc o", p=P)       # [128, 16, 768]
    wo3 = woutT.rearrange("(c p) o -> p c o", p=P)         # [128, 4, 2048]
    out3 = out.rearrange("(tc p) o -> p tc o", p=P)        # [128, 16, 2048]

    with TileCtx(nc, reps) as tc:
        emit_body(nc, tc, hT3, wq3, wo3, cosT, sinT, out3)

    nc.compile()
    return nc


class TileCtx:
    """TileContext wrapper that optionally wraps the body in a repeat loop
    (used only for wall-clock timing; the graded build uses reps=1)."""

    def __init__(self, nc, reps):
        self.nc = nc
        self.reps = reps
        self.tc = tile.TileContext(nc)
        self.loop = None

    def __enter__(self):
        tc = self.tc.__enter__()
        if self.reps > 1:
            self.loop = tc.For_i(0, self.reps, 1)
            self.loop.__enter__()
        return tc

    def __exit__(self, *a):
        if self.loop is not None:
            self.loop.__exit__(*a)
        return self.tc.__exit__(*a)


def weave(a, b):
    """Emit two lists of (cost, fn) steps, interleaved so both streams
    progress proportionally (keeps independent PE work between dependent
    attention steps)."""
    ta = sum(c for c, _ in a) or 1
    tb = sum(c for c, _ in b) or 1
    ia = ib = 0
    ca = cb = 0.0
    while ia < len(a) or ib < len(b):
        fa = ca / ta
        fb = cb / tb
        if ib >= len(b) or (ia < len(a) and fa <= fb):
            c, f = a[ia]
            f()
            ca += c
            ia += 1
        else:
            c, f = b[ib]
            f()
            cb += c
            ib += 1


def emit_body(nc, tc, hT3, wq3, wo3, cosT, sinT, out3):
    with ExitStack() as ctx:
        persist = ctx.enter_context(tc.tile_pool(name="persist", bufs=1))
        qkv = persist.tile([P, OC, S], F16)     # q0..q3, k, v  (qkv^T layout)
        V = persist.tile([P, NKB, HD], F16)     # v in [token, d] layout
        ones = persist.tile([P, P], F16)
        rotm = persist.tile([P, P], F16)        # rotate-half permutation (RT)
        ident = persist.tile([P, P], F16)
        masks = persist.tile([P, P], F16)   # lower-triangular-incl. 0/1 mask
        setup_f32 = persist.tile([P, P], F32)
        cos_sb = persist.tile([P, S], F16)
        sin_sb = persist.tile([P, S], F16)
        nc.scalar.dma_start(out=cos_sb, in_=cosT[:, :])
        nc.scalar.dma_start(out=sin_sb, in_=sinT[:, :])
        # all-ones (denominator matmul lhsT)
        nc.vector.memset(setup_f32, 1.0)
        nc.vector.tensor_copy(out=ones, in_=setup_f32)
        # rotm[p, x] = 1 at x = (p+64) % 128: lhsT of the rotate-half matmul
        nc.gpsimd.memset(setup_f32, 0.0)
        nc.gpsimd.affine_select(   # +1 at x = p + 64 (p < 64)
            out=setup_f32, in_=setup_f32,
            compare_op=mybir.AluOpType.not_equal, fill=1.0,
            base=64, channel_multiplier=1, pattern=[[-1, P]])
        nc.gpsimd.affine_select(   # +1 at x = p - 64 (p >= 64)
            out=setup_f32, in_=setup_f32,
            compare_op=mybir.AluOpType.not_equal, fill=1.0,
            base=-64, channel_multiplier=1, pattern=[[-1, P]])
        nc.vector.tensor_copy(out=rotm, in_=setup_f32)
        make_identity(nc, setup_f32)
        nc.vector.tensor_copy(out=ident, in_=setup_f32)
        # keep 1.0 where q_local >= k_local, else 0
        nc.gpsimd.memset(setup_f32, 1.0)
        nc.gpsimd.affine_select(
            out=setup_f32, in_=setup_f32,
            compare_op=mybir.AluOpType.is_ge,
            fill=0.0,
            base=0,
            channel_multiplier=-1,
            pattern=[[1, P]],
        )
        nc.vector.tensor_copy(out=masks, in_=setup_f32)

        # persistent phase-2/3 inputs, prefetched on the scalar queue
        persist2 = ctx.enter_context(tc.tile_pool(name="persist2", bufs=1))
        attnT = persist2.tile([P, NQ, S], F16)
        wout_sb = persist2.tile([P, NQ, HID], F16)
        nc.scalar.dma_start(out=wout_sb, in_=wo3)

        # ---- long-lived working pools (shared across the woven phases) ----
        wq_pool = ctx.enter_context(tc.tile_pool(name="wq", bufs=1))
        h_pool = ctx.enter_context(tc.tile_pool(name="ht", bufs=2))
        rope_p = ctx.enter_context(tc.tile_pool(name="rope", bufs=2))
        p_pool = ctx.enter_context(tc.tile_pool(name="pp", bufs=18))
        pair_pool = ctx.enter_context(tc.tile_pool(name="pair", bufs=18))
        nrm_pool = ctx.enter_context(tc.tile_pool(name="nrm", bufs=2))
        outp = ctx.enter_context(tc.tile_pool(name="outp", bufs=3))
        # PSUM budget (banks = bufs per tag): qps 2 + rps 1 + ss 2 (scores,
        # shared with the v-transpose tiles) + pso 2 + psd 1 = 8 exactly
        mm_ps = ctx.enter_context(
            tc.tile_pool(name="mmps", bufs=2, space="PSUM"))
        rp_ps = ctx.enter_context(
            tc.tile_pool(name="rpps", bufs=1, space="PSUM"))
        ss_ps = ctx.enter_context(
            tc.tile_pool(name="ssps", bufs=2, space="PSUM"))
        o_ps = ctx.enter_context(
            tc.tile_pool(name="ops", bufs=2, space="PSUM"))
        d_ps = ctx.enter_context(
            tc.tile_pool(name="dps", bufs=1, space="PSUM"))

        wq_sb = wq_pool.tile([P, KC, OC * P], F16)
        # k+v weight cols first (k is the first oc group), then q cols
        for i in range(KC // 2):
            nc.sync.dma_start(out=wq_sb[:, 2 * i:2 * i + 2, NQ * P:],
                              in_=wq3[:, 2 * i:2 * i + 2, NQ * P:])
        ht = {}
        ht[0] = h_pool.tile([P, KC, TB], F16, tag="ht", name="h_t")
        # 4-way split so the first matmuls (subtile deps) start ~1us in
        for i in range(4):
            nc.gpsimd.dma_start(out=ht[0][:, 4 * i:4 * i + 4, :],
                                in_=hT3[:, 4 * i:4 * i + 4, 0:TB])
        for i in range(KC // 2):
            nc.sync.dma_start(out=wq_sb[:, 2 * i:2 * i + 2, :NQ * P],
                              in_=wq3[:, 2 * i:2 * i + 2, :NQ * P])

        inv_sqrt_hd = 1.0 / math.sqrt(HD)
        kT = qkv[:, NQ, :]

        # ---------------- phase-1 steps: qkv^T for one t-block ---------------
        def qkv_steps(tb):
            steps = []
            st = {"deferred": None}
            if tb + 1 < NTB:
                def dma_next():
                    ht[tb + 1] = h_pool.tile([P, KC, TB], F16, tag="ht", name="h_t")
                    nc.gpsimd.dma_start(
                        out=ht[tb + 1],
                        in_=hT3[:, :, (tb + 1) * TB:(tb + 2) * TB])
                steps.append((1, dma_next))

            def run_deferred():
                if st["deferred"] is not None:
                    st["deferred"]()
                    st["deferred"] = None

            def rope_block(oc):
                sl = slice(tb * TB, (tb + 1) * TB)
                ch = qkv[:, oc, sl]
                rps = rp_ps.tile([P, TB], F32, tag="rps")
                nc.tensor.matmul(rps, rotm, ch, start=True, stop=True)
                t1 = rope_p.tile([P, TB], F16, tag="t1")
                nc.vector.tensor_mul(t1, rps, sin_sb[:, sl])
                t2 = rope_p.tile([P, TB], F16, tag="t2")
                nc.gpsimd.tensor_mul(t2, ch, cos_sb[:, sl])
                nc.vector.tensor_add(ch, t1, t2)

            def v_block():
                for i in range(TB // P):
                    kb = tb * (TB // P) + i
                    blk = qkv[:, NQ + 1, kb * P:(kb + 1) * P]
                    vps = ss_ps.tile([P, HD], F16, tag="ss")
                    nc.tensor.transpose(vps, blk, ident)
                    nc.vector.tensor_copy(out=V[:, kb, :], in_=vps)

            for oc in [NQ, NQ + 1, 0, 1, 2, 3]:   # k, v, then q heads
                def mk(oc):
                    def s1():
                        run_deferred()
                        ps = mm_ps.tile([P, TB], F32, tag="qps")
                        st["ps"] = ps
                        for kc in range(KC // 2):
                            nc.tensor.matmul(
                                ps,
                                wq_sb[:, kc, oc * P:(oc + 1) * P],
                                ht[tb][:, kc, :],
                                start=(kc == 0), stop=False)

                    def s2():
                        ps = st["ps"]
                        for kc in range(KC // 2, KC):
                            nc.tensor.matmul(
                                ps,
                                wq_sb[:, kc, oc * P:(oc + 1) * P],
                                ht[tb][:, kc, :],
                                start=False, stop=(kc == KC - 1))
                        # clip(x) = max(min(x, 8), -8), converting to fp16
                        nc.vector.tensor_scalar(
                            out=qkv[:, oc, tb * TB:(tb + 1) * TB],
                            in0=ps,
                            scalar1=CLIP,
                            scalar2=-CLIP,
                            op0=mybir.AluOpType.min,
                            op1=mybir.AluOpType.max,
                        )
                        # rope / v-transpose runs one step later, so the PE
                        # never waits on the DVE clip directly
                        if oc <= NQ:
                            st["deferred"] = lambda: rope_block(oc)
                        else:
                            st["deferred"] = v_block
                    return s1, s2
                s1, s2 = mk(oc)
                steps.append((8 * MM512, s1))
                steps.append((8 * MM512, s2))
            steps.append((MM512, run_deferred))   # tail: last oc's rope
            return steps

        # -------------- phase-2 steps: causal GQA attention, q-block j -------
        # Per (j, h): scores -> exp (ScalarE) -> [mask] -> O^T accumulate,
        # with AV lagging scores by 2 k-blocks so exp latency hides behind
        # the woven filler matmuls. Softmax denominator via an all-ones
        # lhsT matmul over VectorE-pre-accumulated exp tiles; the
        # denominator + normalize for head h are emitted during head h+1.
        def attn_steps(j):
            steps = []
            nk = (j + 1) * DB

            def head_steps(h):
                st = {}
                qT = qkv[:, h, :]

                def start():
                    st["pso"] = o_ps.tile([P, QB], F32, tag="pso", name="pso")
                    st["dtiles"] = []
                    st["prev_p"] = None
                    st["pend_pair"] = None
                    st["offsum"] = None
                    st["acc"] = None
                    st["p"] = {}

                def push_pair(pair):
                    # fold pairs into a single running off-diagonal sum on
                    # VectorE: the PE's softmax-denominator ones-matmul then
                    # sees at most 2 rhs tiles per (j, h)
                    if st["pend_pair"] is None:
                        st["pend_pair"] = pair
                    else:
                        if st["offsum"] is None:
                            quad = pair_pool.tile([P, QB], F16, tag="pr",
                                                  name="quad")
                            nc.vector.tensor_add(quad, st["pend_pair"], pair)
                            st["offsum"] = quad
                            st["dtiles"].append(quad)
                        else:
                            half = pair_pool.tile([P, QB], F16, tag="pr",
                                                  name="half")
                            nc.vector.tensor_add(half, st["pend_pair"], pair)
                            nc.vector.tensor_add(
                                st["offsum"], st["offsum"], half)
                        st["pend_pair"] = None

                def emit_av(kb):
                    r = kb - j * DB
                    q0 = max(0, P * r)
                    nc.tensor.matmul(
                        st["pso"][:, q0:], V[:, kb, :], st["p"][kb][:, q0:],
                        start=(kb == 0), stop=(kb == nk - 1))

                def mk(kb):
                    def f():
                        if kb == 0:
                            start()
                        r = kb - j * DB
                        q0 = max(0, P * r)
                        ps_s = ss_ps.tile([P, QB], F32, tag="ss")
                        nc.tensor.matmul(
                            ps_s[:, q0:],
                            kT[:, kb * P:(kb + 1) * P],
                            qT[:, j * QB + q0:(j + 1) * QB],
                            start=True, stop=True)
                        p_t = p_pool.tile([P, QB], F16, tag="pt")
                        nc.scalar.activation(
                            p_t[:, q0:], ps_s[:, q0:],
                            mybir.ActivationFunctionType.Exp,
                            scale=inv_sqrt_hd)
                        if r >= 0:               # triangular 128-col head
                            # GpSimd: keeps the scores->exp->mask->AV chain
                            # out of the much deeper VectorE queue
                            nc.gpsimd.tensor_mul(
                                p_t[:, q0:q0 + P], p_t[:, q0:q0 + P], masks)
                        st["p"][kb] = p_t
                        # fold exp tiles for the denominator matmul
                        if r < 0:
                            if st["prev_p"] is None:
                                st["prev_p"] = p_t
                            else:
                                pair = pair_pool.tile([P, QB], F16, tag="pr")
                                nc.vector.tensor_add(pair, st["prev_p"], p_t)
                                push_pair(pair)
                                st["prev_p"] = None
                        elif r == 0:
                            # fresh copy: the lag-2 AV still needs p_t intact,
                            # so the denominator accumulation cannot alias it
                            acc = pair_pool.tile([P, QB], F16, tag="pr",
                                                 name="acc")
                            nc.vector.tensor_copy(out=acc, in_=p_t)
                            st["acc"] = acc
                        else:
                            nc.vector.tensor_add(
                                st["acc"][:, q0:], st["acc"][:, q0:],
                                p_t[:, q0:])
                        if kb >= 2:
                            emit_av(kb - 2)
                    return f

                def tail1():
                    emit_av(nk - 2)

                def tail2():
                    emit_av(nk - 1)
                    assert st["prev_p"] is None
                    if st["pend_pair"] is not None:
                        st["dtiles"].append(st["pend_pair"])
                        st["pend_pair"] = None
                    st["dtiles"].append(st["acc"])

                stps = [(2 * MM512, mk(kb)) for kb in range(nk)]
                stps.append((MM512, tail1))
                stps.append((MM512, tail2))
                return stps, st

            def finalize(st, h):
                dtiles = st["dtiles"]
                ps_d = d_ps.tile([P, QB], F32)
                for i, t in enumerate(dtiles):
                    nc.tensor.matmul(ps_d, ones, t,
                                     start=(i == 0),
                                     stop=(i == len(dtiles) - 1))
                recip = nrm_pool.tile([P, QB], F32, tag="recip")
                nc.vector.reciprocal(recip, ps_d)
                nc.vector.tensor_mul(
                    attnT[:, h, j * QB:(j + 1) * QB], st["pso"], recip)

            prev = [None]
            for h in range(NQ):
                hs, st = head_steps(h)
                def mk_fin(h):
                    def f():
                        if prev[0] is not None:
                            finalize(*prev[0])
                        prev[0] = None
                    return f
                # finalize previous head right after this head's first step
                hs.insert(1, (2 * MM512, mk_fin(h)))
                steps.extend(hs)

                def mk_set(st, h):
                    def f():
                        prev[0] = (st, h)
                    return f
                steps.append((1, mk_set(st, h)))

            def last_fin():
                if prev[0] is not None:
                    finalize(*prev[0])
                    prev[0] = None
            steps.append((2 * MM512, last_fin))
            return steps

        # ---------------- phase-3 steps: out = attn_flat @ WoutT -------------
        def out_steps(j, mixed_evac):
            steps = []
            st = {}
            for i in range(DB):
                tci = j * DB + i

                def mk_row(tci):
                    def f():
                        st["orow"] = outp.tile([P, HID], F16, tag="orow", name="orow")
                    return f
                steps.append((1, mk_row(tci)))
                for ob in range(HID // TB):
                    def mk(tci, ob):
                        def f():
                            ps = mm_ps.tile([P, TB], F32, tag="qps")
                            for c in range(NQ):
                                nc.tensor.matmul(
                                    ps,
                                    attnT[:, c, tci * P:(tci + 1) * P],
                                    wout_sb[:, c, ob * TB:(ob + 1) * TB],
                                    start=(c == 0), stop=(c == NQ - 1),
                                )
                            dst = st["orow"][:, ob * TB:(ob + 1) * TB]
                            if mixed_evac and ob % 2 == 0:
                                nc.scalar.copy(out=dst, in_=ps)
                            else:
                                nc.vector.tensor_copy(out=dst, in_=ps)
                            if ob == HID // TB - 1:
                                nc.sync.dma_start(
                                    out=out3[:, tci, :], in_=st["orow"])
                        return f
                    steps.append((4 * MM512, mk(tci, ob)))
            return steps

        # ------------------------- the woven schedule ------------------------
        def run(steps):
            for _, f in steps:
                f()

        run(qkv_steps(0))
        weave(attn_steps(0), qkv_steps(1))
        weave(attn_steps(1), qkv_steps(2) + out_steps(0, False))
        weave(attn_steps(2) + attn_steps(3),
              qkv_steps(3) + out_steps(1, False) + out_steps(2, False))
        run(out_steps(3, True))


def prepare_inputs(hidden_states, position_ids, Wqkv, Wout):
    hidden_states = np.asarray(hidden_states, dtype=np.float32)
    position_ids = np.asarray(position_ids)
    Wqkv = np.asarray(Wqkv, dtype=np.float32)
    Wout = np.asarray(Wout, dtype=np.float32)

    # rope tables, mirroring the reference's f32 math
    inv_freq = (1.0 / (ROPE_THETA ** (np.arange(0, HD, 2, dtype=np.float32)
                                      / np.float32(HD)))).astype(np.float32)
    in_maps = []
    for c in range(NCORES):
        b, g = divmod(c, NQ)
        pos = position_ids[b].astype(np.float32)
        freqs = pos[:, None] * inv_freq[None, :]            # [S, 64] f32
        cos = np.cos(np.concatenate([freqs, freqs], axis=1))  # [S, 128]
        sin = np.sin(np.concatenate([freqs, freqs], axis=1))
        sinS = sin.T.copy()                                  # [128, S]
        sinS[:HD // 2] *= -1.0                               # rotate-half sign
        wq_rows = np.concatenate([
            Wqkv[512 * g:512 * (g + 1)],                     # 4 q heads
            Wqkv[NH * HD + HD * g: NH * HD + HD * (g + 1)],  # k head
            Wqkv[(NH + NKV) * HD + HD * g:
                 (NH + NKV) * HD + HD * (g + 1)],            # v head
        ], axis=0)                                           # [768, HID]
        in_maps.append({
            "hT": np.ascontiguousarray(hidden_states[b].T.astype(np.float16)),
            "wqkvT": np.ascontiguousarray(wq_rows.T.astype(np.float16)),
            "woutT": np.ascontiguousarray(
                Wout[:, 512 * g:512 * (g + 1)].T.astype(np.float16)),
            "cosT": np.ascontiguousarray(cos.T.astype(np.float16)),
            "sinT": np.ascontiguousarray(sinS.astype(np.float16)),
        })
    return in_maps


def assemble(results):
    out = np.zeros((B, S, HID), dtype=np.float64)
    for c in range(NCORES):
        b = c // NQ
        out[b] += results[c]["out"].astype(np.float64)
    return out.astype(np.float32)


_cache = {}


def kernel(hidden_states, position_ids, Wqkv, Wout):
    if "nc" not in _cache:
        _cache["nc"] = build_nc(reps=1)
    nc = _cache["nc"]
    in_maps = prepare_inputs(hidden_states, position_ids, Wqkv, Wout)
    res = run_bass_kernel_spmd(nc, in_maps, core_ids=list(range(NCORES)))
    return assemble(res.results)
